# revision 18
# baseline (speedup 1.0000x reference)
"""Trainium2 Bass kernel for the spiking-dense first-crossing problem.

Computes out[n,y] = min(1 + argmax_t(V[t,n,y] > 1), 64) where
V[t] = (spike mask up to t) @ weight, via one big masked matmul:

  V^T[(y), (n,t)] = W_slice^T @ mask   (W stationary, y on PSUM partitions)

All matmul operands are bf16: the {0,1} mask is exact in bf16, so the
only quantization is the weight rounding (~2^-9 rel). Elements whose
|V-1| margin is below FIX_EPS are recomputed exactly on host from the
margin plane the kernel also emits.

The mask is static per call, so it is built host-side and streamed in
with the weights: each 128-row contraction chunk is one combined
[w_chunk | mask_chunk] DMA. This keeps the DVE free for postproc (an
on-chip is_ge mask build was ~27us of DVE busy, co-critical with the
PE) and lets the PE start as soon as chunk 0 lands.

Postproc per PSUM tile: ACT u=V-1 (single PSUM read), DVE b=(u>0),
z=b*(T-t), max-reduce -> crossing code; min-|u|-reduce -> margin.

Sharding: 2-way over Y (output cols) x 4-way over batch N across the 8
NeuronCores; each core computes a (1024 y, 16 n) block of out^T. The
full weight column-slice (2048 x 1024, bf16) stays resident in SBUF.
"""
import os
import sys
import numpy as np

for _p in ('/opt/trn_rl_repo',):
    if os.path.isdir(_p) and _p not in sys.path:
        sys.path.append(_p)

X, T, NN, YY = 2048, 64, 64, 2048
Y_SH, N_SH = 2, 4
YC = YY // Y_SH          # 1024 y-cols per core
NCB = NN // N_SH         # 16 batch rows per core
KC = X // 128            # 16 contraction chunks
NPF = 8                  # n's per f-half
TT = T - 1               # t=63 crossing gives out=64, same as none
FT = NCB * TT            # 1008 mask free cols per core
FH = NPF * TT            # 504 cols per f-half
NYT = YC // 128          # 8 y-tiles

MODE = os.environ.get("SPIKE_MM_MODE", "bf16")  # bf16 | fp8dr
FP8 = (MODE == "fp8dr")
KCC = KC // 2 if FP8 else KC    # contraction chunks per instruction stream
WSCALE = 8.0 if FP8 else 1.0    # w pre-scale: keeps e4m3 out of denormals
# host-recompute elements with |V-1| margin below this
FIX_EPS = 0.12 if FP8 else 8e-3
TRACE = False

_cache = {}
LAST_RESULTS = None


def _ensure_ntff_hook():
    """Register the axon NTFF profiling hook if the environment lacks
    antenv.axon_hooks (the slim agent image) but has trn_agent_boot.
    Only adds capability; no-op when the real module exists."""
    try:
        import antenv.axon_hooks  # noqa: F401
        return
    except ImportError:
        pass
    try:
        import types
        from trn_agent_boot.trn_boot import _ntff_profile_via_ctypes
        hook = _ntff_profile_via_ctypes('/opt/axon/libaxon_pjrt.so')
        if hook is None:
            return
        import antenv
        mod = types.ModuleType('antenv.axon_hooks')
        mod.get_axon_ntff_profile_hook = lambda: hook
        mod.set_axon_ntff_profile_hook = lambda h: None
        sys.modules['antenv.axon_hooks'] = mod
        antenv.axon_hooks = mod
    except Exception:
        pass


def _safe_upload_artifacts():
    """upload_artifacts needs a bucket; make it degrade to a no-op path
    so tracing works in sandboxes without one."""
    try:
        from concourse import bass_utils
        orig = bass_utils.upload_artifacts
        if getattr(bass_utils, "_ul_wrapped", False):
            return
        def wrapped(tmpdir):
            try:
                return orig(tmpdir)
            except Exception:
                return str(tmpdir)
        bass_utils.upload_artifacts = wrapped
        bass_utils._ul_wrapped = True
    except Exception:
        pass


def _build_nc(reps=1):
    import concourse.bacc as bacc
    import concourse.mybir as mybir
    import concourse.tile as tile

    dt = mybir.dt
    f32 = dt.float32
    bf16 = dt.bfloat16
    nc = bacc.Bacc("TRN2", target_bir_lowering=False, debug=False)

    # per-chunk rows: [weight cols | mask cols]; fp8dr interleaves two
    # 128-row contraction planes per chunk for DoubleRow (K=256/matmul)
    mdt = dt.float8e4 if FP8 else bf16
    CW = 2 * (YC + FT) if FP8 else YC + FT
    WOFF = 2 * YC if FP8 else YC
    wm_d = nc.dram_tensor("wm", (KCC * 128, CW), mdt,
                          kind="ExternalInput")
    aux_d = nc.dram_tensor("aux", (128, TT), bf16, kind="ExternalInput")
    out_d = nc.dram_tensor("out", (YC, NCB), f32, kind="ExternalOutput")
    marg_d = nc.dram_tensor("marg", (YC, NCB), bf16, kind="ExternalOutput")

    with tile.TileContext(nc) as tc:
        with tc.tile_pool(name="const", bufs=1) as cpool, \
             tc.tile_pool(name="wp", bufs=1) as wpool, \
             tc.tile_pool(name="ps", bufs=8, space="PSUM") as ps, \
             tc.tile_pool(name="uz", bufs=8) as uzpool, \
             tc.tile_pool(name="sm", bufs=8) as smpool, \
             tc.tile_pool(name="po", bufs=8) as popool:
            neg1_sb = cpool.tile([128, 1], f32, tag="neg1")
            nc.vector.memset(neg1_sb, -1.0)
            junk_sb = cpool.tile([128, 512], bf16, tag="junk")
            nc.vector.memset(junk_sb, 1.0)

            for rep in range(reps):
                # combined weight|mask chunks, resident; chunk 0 issued
                # first since it gates the first real matmul
                # The DMA engines interleave ~2 in-flight descriptors
                # per queue, so early chunks are split into consecutive
                # pieces: the lookahead then works on one chunk instead of
                # stealing its bandwidth for the next one.
                wm_tiles = [wpool.tile([128, CW], mdt, tag=f"wm{k}",
                                       name=f"wm{k}")
                            for k in range(KCC)]
                q4 = CW // 4
                for piece in range(2):      # chunk 0 as 2 quarters per queue
                    nc.sync.dma_start(
                        out=wm_tiles[0][:, piece * q4:(piece + 1) * q4],
                        in_=wm_d.ap()[0:128, piece * q4:(piece + 1) * q4])
                    nc.scalar.dma_start(
                        out=wm_tiles[0][:, (2 + piece) * q4:(3 + piece) * q4],
                        in_=wm_d.ap()[0:128, (2 + piece) * q4:(3 + piece) * q4])
                nc.sync.dma_start(          # chunk 1 as a half per queue
                    out=wm_tiles[1][:, 0:CW // 2],
                    in_=wm_d.ap()[128:256, 0:CW // 2])
                nc.scalar.dma_start(
                    out=wm_tiles[1][:, CW // 2:CW],
                    in_=wm_d.ap()[128:256, CW // 2:CW])
                for k in range(2, KCC):
                    eng = nc.sync if k % 2 == 0 else nc.scalar
                    eng.dma_start(out=wm_tiles[k],
                                  in_=wm_d.ap()[k * 128:(k + 1) * 128, :])
                aux_sb = cpool.tile([128, TT], bf16, tag="aux")
                nc.sync.dma_start(out=aux_sb, in_=aux_d.ap())
                revt_sb = aux_sb[:, 0:TT]

                # PE warmup: junk matmuls keep the PE busy through the
                # startup DMA window so HAM un-throttles before the first
                # real matmul arrives.
                warm_pt = ps.tile([128, 512], f32, tag="pt", name="warm_pt")
                for _ in range(4):
                    nc.tensor.matmul(warm_pt, junk_sb[:, 0:128], junk_sb[:],
                                     start=True, stop=True)

                rm_tiles = [smpool.tile([128, NCB], bf16, tag="rm",
                                        name=f"rm{yt}")
                            for yt in range(NYT)]
                mg_tiles = [smpool.tile([128, NCB], bf16, tag="mg",
                                        name=f"mg{yt}")
                            for yt in range(NYT)]

                def emit_mm(pt, k, yt, fs, fe):
                    wmt = wm_tiles[k]
                    if FP8:
                        lhsT = wmt[:, 0:2 * YC] \
                            .rearrange("p (i y) -> p i y", i=2) \
                            [:, :, yt * 128:(yt + 1) * 128]
                        rhs = wmt[:, 2 * YC:CW] \
                            .rearrange("p (i c) -> p i c", i=2)[:, :, fs:fe]
                        nc.tensor.matmul(
                            pt, lhsT, rhs,
                            start=(k == 0), stop=(k == KCC - 1),
                            perf_mode=mybir.MatmulPerfMode.DoubleRow)
                    else:
                        nc.tensor.matmul(pt,
                                         wmt[:, yt * 128:(yt + 1) * 128],
                                         wmt[:, YC + fs:YC + fe],
                                         start=(k == 0), stop=(k == KCC - 1))

                def emit_post(pt, yt, cs, width):
                    # cs: first n-col in rm/mg tiles; width: n count
                    nelem = width * TT
                    u_t = uzpool.tile([128, 512], bf16, tag="u")
                    u = u_t[:, 0:nelem]
                    nc.scalar.activation(u, pt,
                                         mybir.ActivationFunctionType.Copy,
                                         bias=-WSCALE)
                    b_t = uzpool.tile([128, 512], bf16, tag="b")
                    b = b_t[:, 0:nelem]
                    nc.vector.tensor_scalar(b, u, 0.0, None,
                                            mybir.AluOpType.is_gt)
                    z_t = uzpool.tile([128, 512], bf16, tag="z")
                    z = z_t[:, 0:nelem]
                    r_b = revt_sb.unsqueeze(1).broadcast_to((128, width, TT))
                    nc.vector.tensor_tensor(
                        z.rearrange("p (n t) -> p n t", n=width),
                        b.rearrange("p (n t) -> p n t", n=width),
                        r_b, mybir.AluOpType.mult)
                    nc.vector.tensor_reduce(
                        rm_tiles[yt][:, cs:cs + width],
                        z.rearrange("p (n t) -> p n t", n=width),
                        axis=mybir.AxisListType.X, op=mybir.AluOpType.max)
                    nc.vector.tensor_reduce(
                        mg_tiles[yt][:, cs:cs + width],
                        u.rearrange("p (n t) -> p n t", n=width),
                        axis=mybir.AxisListType.X, op=mybir.AluOpType.min,
                        apply_absolute_value=True)

                def emit_tail(yt):
                    tmp_t = popool.tile([128, NCB], bf16, tag="tmp")
                    nc.scalar.activation(tmp_t, rm_tiles[yt],
                                         mybir.ActivationFunctionType.Relu,
                                         bias=neg1_sb[:])
                    out_t = popool.tile([128, NCB], f32, tag="pout")
                    nc.scalar.activation(out_t, tmp_t,
                                         mybir.ActivationFunctionType.Copy,
                                         bias=float(T), scale=-1.0)
                    nc.sync.dma_start(
                        out=marg_d.ap()[yt * 128:(yt + 1) * 128, :],
                        in_=mg_tiles[yt])
                    nc.sync.dma_start(
                        out=out_d.ap()[yt * 128:(yt + 1) * 128, :],
                        in_=out_t)

                # f0 pass: k-outer so the PE trails the mask builder
                # without stalling; all 8 banks accumulate in parallel.
                pts = []
                for k in range(KCC):
                    for yt in range(NYT):
                        if k == 0:
                            pts.append(ps.tile([128, FH], f32, tag="pt",
                                               name=f"pt0_{yt}"))
                        emit_mm(pts[yt], k, yt, 0, FH)
                for yt in range(NYT):
                    emit_post(pts[yt], yt, 0, NPF)

                # f1 pass: masks resident -> y-outer so banks finish
                # staggered and postproc overlaps later y-tiles. The last
                # y-tile runs as two 256-col halves to shorten the final
                # exposed drain chain.
                for yt in range(NYT - 1):
                    pt = ps.tile([128, FH], f32, tag="pt", name=f"pt1_{yt}")
                    for k in range(KCC):
                        emit_mm(pt, k, yt, FH, 2 * FH)
                    emit_post(pt, yt, NPF, NPF)
                    emit_tail(yt)
                yt = NYT - 1
                pieces = [(FH, FH // 2, NPF, 4),
                          (FH + FH // 2, FH // 4, NPF + 4, 2),
                          (FH + 3 * (FH // 4), FH // 4, NPF + 6, 2)]
                for pi, (fs, fw, cs, width) in enumerate(pieces):
                    pth = ps.tile([128, FH // 2], f32, tag="pt",
                                  name=f"pt1h{pi}")
                    for k in range(KCC):
                        emit_mm(pth[:, 0:fw], k, yt, fs, fs + fw)
                    emit_post(pth[:, 0:fw], yt, cs, width)
                emit_tail(yt)

    nc.compile()
    return nc


def _make_in_maps(inputs):
    import ml_dtypes

    input = np.ascontiguousarray(np.asarray(inputs["input"], dtype=np.float32))
    weight = np.ascontiguousarray(np.asarray(inputs["weight"], dtype=np.float32))

    # mask[x, n*TT+t] = (t >= s[n, x]); {0,1} is exact in bf16. t=63 is
    # dropped: a first crossing there yields out=64, same as none.
    tgrid = np.arange(TT, dtype=np.float32)
    REVT = np.tile(np.float32(T) - tgrid, (128, 1)).astype(ml_dtypes.bfloat16)

    mmdt = ml_dtypes.float8_e4m3 if FP8 else ml_dtypes.bfloat16

    def plane_pack(a):
        # rows x -> (chunk kk, plane i, partition p); fp8dr packs the two
        # 128-row planes of each K=256 chunk side by side per partition
        if not FP8:
            return a
        c = a.shape[1]
        return np.ascontiguousarray(
            a.reshape(KCC, 2, 128, c).transpose(0, 2, 1, 3)
            .reshape(KCC * 128, 2 * c))

    w_halves = [
        plane_pack((weight[:, yb * YC:(yb + 1) * YC] * WSCALE).astype(mmdt))
        for yb in range(Y_SH)
    ]
    mask_quads = []
    for nb in range(N_SH):
        insl = input[nb * NCB:(nb + 1) * NCB, :]          # (NCB, X)
        m = (tgrid[None, None, :] >= insl[:, :, None])    # (NCB, X, TT)
        m = m.transpose(1, 0, 2).reshape(X, FT).astype(mmdt)
        mask_quads.append(plane_pack(m))

    in_maps = []
    for c in range(8):
        yb, nb = c % Y_SH, c // Y_SH
        wm = np.concatenate([w_halves[yb], mask_quads[nb]], axis=1)
        in_maps.append({"wm": np.ascontiguousarray(wm),
                        "aux": np.ascontiguousarray(REVT)})
    return in_maps


def kernel(input, weight, t_series, T=64, **unused):
    global LAST_RESULTS
    from concourse import bass_utils

    _ensure_ntff_hook()
    _safe_upload_artifacts()
    if "nc" not in _cache:
        _cache["nc"] = _build_nc()
    nc = _cache["nc"]

    _cache["t_series"] = np.asarray(t_series, dtype=np.float32).reshape(-1)
    in_maps = _make_in_maps({"input": input, "weight": weight})

    res = bass_utils.run_bass_kernel_spmd(
        nc, in_maps, core_ids=list(range(8)), trace=TRACE)
    LAST_RESULTS = res

    O = np.empty((YY, NN), dtype=np.float32)
    M = np.empty((YY, NN), dtype=np.float32)
    for c, r in enumerate(res.results):
        yb, nb = c % Y_SH, c // Y_SH
        O[yb * YC:(yb + 1) * YC, nb * NCB:(nb + 1) * NCB] = r["out"]
        M[yb * YC:(yb + 1) * YC, nb * NCB:(nb + 1) * NCB] = \
            np.asarray(r["marg"]).astype(np.float32)
    out = np.ascontiguousarray(O.T)

    _host_fixup(out, M.T, np.asarray(input, np.float32),
                np.asarray(weight, np.float32))
    return out


def _host_fixup(out, margin, input, weight):
    """Recompute exactly (fp64) every element whose bf16 |V-1| margin is
    within the bf16 matmul error bound; in-place on `out`."""
    flags = margin < FIX_EPS * WSCALE   # device margin is in scaled-V units
    if not flags.any():
        return
    # first step index j with t_series[j] >= in; == T means never spikes
    s = np.searchsorted(_cache.get("t_series", np.arange(T, dtype=np.float32)),
                        input, side="left").astype(np.int64)
    s = np.clip(s, 0, T)
    w64 = weight.astype(np.float64)
    for n in np.unique(np.nonzero(flags)[0]):
        ys = np.nonzero(flags[n])[0]
        d = np.zeros((T + 1, len(ys)))
        np.add.at(d, s[n], w64[:, ys])           # scatter rows by spike step
        V = np.cumsum(d[:T], axis=0)
        c = V > 1.0
        any_c = c.any(axis=0)
        idx = np.argmax(c, axis=0)
        out[n, ys] = np.where(any_c, idx + 1, T).astype(np.float32)


# revision 19
# speedup vs baseline: 1.0260x; 1.0260x over previous
"""Trainium2 Bass kernel for the spiking-dense first-crossing problem.

Computes out[n,y] = min(1 + argmax_t(V[t,n,y] > 1), 64) where
V[t] = (spike mask up to t) @ weight, via one big masked matmul:

  V^T[(y), (n,t)] = W_slice^T @ mask   (W stationary, y on PSUM partitions)

All matmul operands are bf16: the {0,1} mask is exact in bf16, so the
only quantization is the weight rounding (~2^-9 rel). Elements whose
|V-1| margin is below FIX_EPS are recomputed exactly on host from the
margin plane the kernel also emits.

The mask is static per call, so it is built host-side and streamed in
with the weights: each 128-row contraction chunk is one combined
[w_chunk | mask_chunk] DMA. This keeps the DVE free for postproc (an
on-chip is_ge mask build was ~27us of DVE busy, co-critical with the
PE) and lets the PE start as soon as chunk 0 lands.

Postproc per PSUM tile: ACT u=V-1 (single PSUM read), DVE b=(u>0),
z=b*(T-t), max-reduce -> crossing code; min-|u|-reduce -> margin.

Sharding: 2-way over Y (output cols) x 4-way over batch N across the 8
NeuronCores; each core computes a (1024 y, 16 n) block of out^T. The
full weight column-slice (2048 x 1024, bf16) stays resident in SBUF.
"""
import os
import sys
import numpy as np

for _p in ('/opt/trn_rl_repo',):
    if os.path.isdir(_p) and _p not in sys.path:
        sys.path.append(_p)

X, T, NN, YY = 2048, 64, 64, 2048
Y_SH, N_SH = 2, 4
YC = YY // Y_SH          # 1024 y-cols per core
NCB = NN // N_SH         # 16 batch rows per core
KC = X // 128            # 16 contraction chunks
NPF = 8                  # n's per f-half
TT = T - 1               # t=63 crossing gives out=64, same as none
FT = NCB * TT            # 1008 mask free cols per core
FH = NPF * TT            # 504 cols per f-half
NYT = YC // 128          # 8 y-tiles

MODE = os.environ.get("SPIKE_MM_MODE", "bf16")  # bf16 | fp8dr
FP8 = (MODE == "fp8dr")
KCC = KC // 2 if FP8 else KC    # contraction chunks per instruction stream
WSCALE = 8.0 if FP8 else 1.0    # w pre-scale: keeps e4m3 out of denormals
# host-recompute elements with |V-1| margin below this
FIX_EPS = 0.12 if FP8 else 8e-3
TRACE = False

_cache = {}
LAST_RESULTS = None


def _ensure_ntff_hook():
    """Register the axon NTFF profiling hook if the environment lacks
    antenv.axon_hooks (the slim agent image) but has trn_agent_boot.
    Only adds capability; no-op when the real module exists."""
    try:
        import antenv.axon_hooks  # noqa: F401
        return
    except ImportError:
        pass
    try:
        import types
        from trn_agent_boot.trn_boot import _ntff_profile_via_ctypes
        hook = _ntff_profile_via_ctypes('/opt/axon/libaxon_pjrt.so')
        if hook is None:
            return
        import antenv
        mod = types.ModuleType('antenv.axon_hooks')
        mod.get_axon_ntff_profile_hook = lambda: hook
        mod.set_axon_ntff_profile_hook = lambda h: None
        sys.modules['antenv.axon_hooks'] = mod
        antenv.axon_hooks = mod
    except Exception:
        pass


def _safe_upload_artifacts():
    """upload_artifacts needs a bucket; make it degrade to a no-op path
    so tracing works in sandboxes without one."""
    try:
        from concourse import bass_utils
        orig = bass_utils.upload_artifacts
        if getattr(bass_utils, "_ul_wrapped", False):
            return
        def wrapped(tmpdir):
            try:
                return orig(tmpdir)
            except Exception:
                return str(tmpdir)
        bass_utils.upload_artifacts = wrapped
        bass_utils._ul_wrapped = True
    except Exception:
        pass


def _build_nc(reps=1):
    import concourse.bacc as bacc
    import concourse.mybir as mybir
    import concourse.tile as tile

    dt = mybir.dt
    f32 = dt.float32
    bf16 = dt.bfloat16
    nc = bacc.Bacc("TRN2", target_bir_lowering=False, debug=False)

    # per-chunk rows: [weight cols | mask cols]; fp8dr interleaves two
    # 128-row contraction planes per chunk for DoubleRow (K=256/matmul)
    mdt = dt.float8e4 if FP8 else bf16
    CW = 2 * (YC + FT) if FP8 else YC + FT
    WOFF = 2 * YC if FP8 else YC
    wm_d = nc.dram_tensor("wm", (KCC * 128, CW), mdt,
                          kind="ExternalInput")
    aux_d = nc.dram_tensor("aux", (128, TT), bf16, kind="ExternalInput")
    out_d = nc.dram_tensor("out", (YC, NCB), f32, kind="ExternalOutput")
    marg_d = nc.dram_tensor("marg", (YC, NCB), bf16, kind="ExternalOutput")

    with tile.TileContext(nc) as tc:
        with tc.tile_pool(name="const", bufs=1) as cpool, \
             tc.tile_pool(name="wp", bufs=1) as wpool, \
             tc.tile_pool(name="ps", bufs=8, space="PSUM") as ps, \
             tc.tile_pool(name="uz", bufs=8) as uzpool, \
             tc.tile_pool(name="sm", bufs=8) as smpool, \
             tc.tile_pool(name="po", bufs=8) as popool:
            neg1_sb = cpool.tile([128, 1], f32, tag="neg1")
            nc.vector.memset(neg1_sb, -1.0)
            junk_sb = cpool.tile([128, 512], bf16, tag="junk")
            nc.vector.memset(junk_sb, 1.0)

            for rep in range(reps):
                # combined weight|mask chunks, resident; chunk 0 issued
                # first since it gates the first real matmul
                # Whole-chunk descriptors arrive soonest (splitting
                # adds per-descriptor latency); stripe across both HWDGE
                # queues so aggregate bandwidth covers the k-outer pace.
                wm_tiles = [wpool.tile([128, CW], mdt, tag=f"wm{k}",
                                       name=f"wm{k}")
                            for k in range(KCC)]
                for k in range(KCC):
                    eng = nc.sync if k % 2 == 0 else nc.scalar
                    eng.dma_start(out=wm_tiles[k],
                                  in_=wm_d.ap()[k * 128:(k + 1) * 128, :])
                aux_sb = cpool.tile([128, TT], bf16, tag="aux")
                nc.sync.dma_start(out=aux_sb, in_=aux_d.ap())
                revt_sb = aux_sb[:, 0:TT]

                # PE warmup: junk matmuls keep the PE busy through the
                # startup DMA window so HAM un-throttles before the first
                # real matmul arrives.
                warm_pt = ps.tile([128, 512], f32, tag="pt", name="warm_pt")
                for _ in range(8):
                    nc.tensor.matmul(warm_pt, junk_sb[:, 0:128], junk_sb[:],
                                     start=True, stop=True)

                rm_tiles = [smpool.tile([128, NCB], bf16, tag="rm",
                                        name=f"rm{yt}")
                            for yt in range(NYT)]
                mg_tiles = [smpool.tile([128, NCB], bf16, tag="mg",
                                        name=f"mg{yt}")
                            for yt in range(NYT)]

                def emit_mm(pt, k, yt, fs, fe):
                    wmt = wm_tiles[k]
                    if FP8:
                        lhsT = wmt[:, 0:2 * YC] \
                            .rearrange("p (i y) -> p i y", i=2) \
                            [:, :, yt * 128:(yt + 1) * 128]
                        rhs = wmt[:, 2 * YC:CW] \
                            .rearrange("p (i c) -> p i c", i=2)[:, :, fs:fe]
                        nc.tensor.matmul(
                            pt, lhsT, rhs,
                            start=(k == 0), stop=(k == KCC - 1),
                            perf_mode=mybir.MatmulPerfMode.DoubleRow)
                    else:
                        nc.tensor.matmul(pt,
                                         wmt[:, yt * 128:(yt + 1) * 128],
                                         wmt[:, YC + fs:YC + fe],
                                         start=(k == 0), stop=(k == KCC - 1))

                def emit_post(pt, yt, cs, width):
                    # cs: first n-col in rm/mg tiles; width: n count
                    nelem = width * TT
                    u_t = uzpool.tile([128, 512], bf16, tag="u")
                    u = u_t[:, 0:nelem]
                    nc.scalar.activation(u, pt,
                                         mybir.ActivationFunctionType.Copy,
                                         bias=-WSCALE)
                    b_t = uzpool.tile([128, 512], bf16, tag="b")
                    b = b_t[:, 0:nelem]
                    nc.vector.tensor_scalar(b, u, 0.0, None,
                                            mybir.AluOpType.is_gt)
                    z_t = uzpool.tile([128, 512], bf16, tag="z")
                    z = z_t[:, 0:nelem]
                    r_b = revt_sb.unsqueeze(1).broadcast_to((128, width, TT))
                    nc.vector.tensor_tensor(
                        z.rearrange("p (n t) -> p n t", n=width),
                        b.rearrange("p (n t) -> p n t", n=width),
                        r_b, mybir.AluOpType.mult)
                    nc.vector.tensor_reduce(
                        rm_tiles[yt][:, cs:cs + width],
                        z.rearrange("p (n t) -> p n t", n=width),
                        axis=mybir.AxisListType.X, op=mybir.AluOpType.max)
                    nc.vector.tensor_reduce(
                        mg_tiles[yt][:, cs:cs + width],
                        u.rearrange("p (n t) -> p n t", n=width),
                        axis=mybir.AxisListType.X, op=mybir.AluOpType.min,
                        apply_absolute_value=True)

                def emit_tail(yt):
                    tmp_t = popool.tile([128, NCB], bf16, tag="tmp")
                    nc.scalar.activation(tmp_t, rm_tiles[yt],
                                         mybir.ActivationFunctionType.Relu,
                                         bias=neg1_sb[:])
                    out_t = popool.tile([128, NCB], f32, tag="pout")
                    nc.scalar.activation(out_t, tmp_t,
                                         mybir.ActivationFunctionType.Copy,
                                         bias=float(T), scale=-1.0)
                    nc.sync.dma_start(
                        out=marg_d.ap()[yt * 128:(yt + 1) * 128, :],
                        in_=mg_tiles[yt])
                    nc.sync.dma_start(
                        out=out_d.ap()[yt * 128:(yt + 1) * 128, :],
                        in_=out_t)

                # f0 pass: k-outer so the PE trails the mask builder
                # without stalling; all 8 banks accumulate in parallel.
                pts = []
                for k in range(KCC):
                    for yt in range(NYT):
                        if k == 0:
                            pts.append(ps.tile([128, FH], f32, tag="pt",
                                               name=f"pt0_{yt}"))
                        emit_mm(pts[yt], k, yt, 0, FH)
                for yt in range(NYT):
                    emit_post(pts[yt], yt, 0, NPF)

                # f1 pass: masks resident -> y-outer so banks finish
                # staggered and postproc overlaps later y-tiles. The last
                # y-tile runs as two 256-col halves to shorten the final
                # exposed drain chain.
                for yt in range(NYT - 1):
                    pt = ps.tile([128, FH], f32, tag="pt", name=f"pt1_{yt}")
                    for k in range(KCC):
                        emit_mm(pt, k, yt, FH, 2 * FH)
                    emit_post(pt, yt, NPF, NPF)
                    emit_tail(yt)
                yt = NYT - 1
                for half in range(2):
                    pth = ps.tile([128, FH // 2], f32, tag="pt",
                                  name=f"pt1h{half}")
                    fs = FH + half * (FH // 2)
                    for k in range(KCC):
                        emit_mm(pth, k, yt, fs, fs + FH // 2)
                    emit_post(pth, yt, NPF + half * 4, 4)
                emit_tail(yt)

    nc.compile()
    return nc


def _make_in_maps(inputs):
    import ml_dtypes

    input = np.ascontiguousarray(np.asarray(inputs["input"], dtype=np.float32))
    weight = np.ascontiguousarray(np.asarray(inputs["weight"], dtype=np.float32))

    # mask[x, n*TT+t] = (t >= s[n, x]); {0,1} is exact in bf16. t=63 is
    # dropped: a first crossing there yields out=64, same as none.
    tgrid = np.arange(TT, dtype=np.float32)
    REVT = np.tile(np.float32(T) - tgrid, (128, 1)).astype(ml_dtypes.bfloat16)

    mmdt = ml_dtypes.float8_e4m3 if FP8 else ml_dtypes.bfloat16

    def plane_pack(a):
        # rows x -> (chunk kk, plane i, partition p); fp8dr packs the two
        # 128-row planes of each K=256 chunk side by side per partition
        if not FP8:
            return a
        c = a.shape[1]
        return np.ascontiguousarray(
            a.reshape(KCC, 2, 128, c).transpose(0, 2, 1, 3)
            .reshape(KCC * 128, 2 * c))

    w_halves = [
        plane_pack((weight[:, yb * YC:(yb + 1) * YC] * WSCALE).astype(mmdt))
        for yb in range(Y_SH)
    ]
    mask_quads = []
    for nb in range(N_SH):
        insl = input[nb * NCB:(nb + 1) * NCB, :]          # (NCB, X)
        m = (tgrid[None, None, :] >= insl[:, :, None])    # (NCB, X, TT)
        m = m.transpose(1, 0, 2).reshape(X, FT).astype(mmdt)
        mask_quads.append(plane_pack(m))

    in_maps = []
    for c in range(8):
        yb, nb = c % Y_SH, c // Y_SH
        wm = np.concatenate([w_halves[yb], mask_quads[nb]], axis=1)
        in_maps.append({"wm": np.ascontiguousarray(wm),
                        "aux": np.ascontiguousarray(REVT)})
    return in_maps


def kernel(input, weight, t_series, T=64, **unused):
    global LAST_RESULTS
    from concourse import bass_utils

    _ensure_ntff_hook()
    _safe_upload_artifacts()
    if "nc" not in _cache:
        _cache["nc"] = _build_nc()
    nc = _cache["nc"]

    _cache["t_series"] = np.asarray(t_series, dtype=np.float32).reshape(-1)
    in_maps = _make_in_maps({"input": input, "weight": weight})

    res = bass_utils.run_bass_kernel_spmd(
        nc, in_maps, core_ids=list(range(8)), trace=TRACE)
    LAST_RESULTS = res

    O = np.empty((YY, NN), dtype=np.float32)
    M = np.empty((YY, NN), dtype=np.float32)
    for c, r in enumerate(res.results):
        yb, nb = c % Y_SH, c // Y_SH
        O[yb * YC:(yb + 1) * YC, nb * NCB:(nb + 1) * NCB] = r["out"]
        M[yb * YC:(yb + 1) * YC, nb * NCB:(nb + 1) * NCB] = \
            np.asarray(r["marg"]).astype(np.float32)
    out = np.ascontiguousarray(O.T)

    _host_fixup(out, M.T, np.asarray(input, np.float32),
                np.asarray(weight, np.float32))
    return out


def _host_fixup(out, margin, input, weight):
    """Recompute exactly (fp64) every element whose bf16 |V-1| margin is
    within the bf16 matmul error bound; in-place on `out`."""
    flags = margin < FIX_EPS * WSCALE   # device margin is in scaled-V units
    if not flags.any():
        return
    # first step index j with t_series[j] >= in; == T means never spikes
    s = np.searchsorted(_cache.get("t_series", np.arange(T, dtype=np.float32)),
                        input, side="left").astype(np.int64)
    s = np.clip(s, 0, T)
    w64 = weight.astype(np.float64)
    for n in np.unique(np.nonzero(flags)[0]):
        ys = np.nonzero(flags[n])[0]
        d = np.zeros((T + 1, len(ys)))
        np.add.at(d, s[n], w64[:, ys])           # scatter rows by spike step
        V = np.cumsum(d[:T], axis=0)
        c = V > 1.0
        any_c = c.any(axis=0)
        idx = np.argmax(c, axis=0)
        out[n, ys] = np.where(any_c, idx + 1, T).astype(np.float32)


# revision 20
# speedup vs baseline: 1.0307x; 1.0046x over previous
"""Trainium2 Bass kernel for the spiking-dense first-crossing problem.

Computes out[n,y] = min(1 + argmax_t(V[t,n,y] > 1), 64) where
V[t] = (spike mask up to t) @ weight, via one big masked matmul:

  V^T[(y), (n,t)] = W_slice^T @ mask   (W stationary, y on PSUM partitions)

All matmul operands are bf16: the {0,1} mask is exact in bf16, so the
only quantization is the weight rounding (~2^-9 rel). Elements whose
|V-1| margin is below FIX_EPS are recomputed exactly on host from the
margin plane the kernel also emits.

The mask is static per call, so it is built host-side and streamed in
with the weights: each 128-row contraction chunk is one combined
[w_chunk | mask_chunk] DMA. This keeps the DVE free for postproc (an
on-chip is_ge mask build was ~27us of DVE busy, co-critical with the
PE) and lets the PE start as soon as chunk 0 lands.

Postproc per PSUM tile: ACT u=V-1 (single PSUM read), DVE b=(u>0),
z=b*(T-t), max-reduce -> crossing code; min-|u|-reduce -> margin.

Sharding: 2-way over Y (output cols) x 4-way over batch N across the 8
NeuronCores; each core computes a (1024 y, 16 n) block of out^T. The
full weight column-slice (2048 x 1024, bf16) stays resident in SBUF.
"""
import os
import sys
import numpy as np

for _p in ('/opt/trn_rl_repo',):
    if os.path.isdir(_p) and _p not in sys.path:
        sys.path.append(_p)

X, T, NN, YY = 2048, 64, 64, 2048
Y_SH, N_SH = 2, 4
YC = YY // Y_SH          # 1024 y-cols per core
NCB = NN // N_SH         # 16 batch rows per core
KC = X // 128            # 16 contraction chunks
NPF = 8                  # n's per f-half
TT = T - 1               # t=63 crossing gives out=64, same as none
FT = NCB * TT            # 1008 mask free cols per core
FH = NPF * TT            # 504 cols per f-half
NYT = YC // 128          # 8 y-tiles

MODE = os.environ.get("SPIKE_MM_MODE", "bf16")  # bf16 | fp8dr
FP8 = (MODE == "fp8dr")
KCC = KC // 2 if FP8 else KC    # contraction chunks per instruction stream
WSCALE = 8.0 if FP8 else 1.0    # w pre-scale: keeps e4m3 out of denormals
# host-recompute elements with |V-1| margin below this
FIX_EPS = 0.12 if FP8 else 8e-3
TRACE = False

_cache = {}
LAST_RESULTS = None


def _ensure_ntff_hook():
    """Register the axon NTFF profiling hook if the environment lacks
    antenv.axon_hooks (the slim agent image) but has trn_agent_boot.
    Only adds capability; no-op when the real module exists."""
    try:
        import antenv.axon_hooks  # noqa: F401
        return
    except ImportError:
        pass
    try:
        import types
        from trn_agent_boot.trn_boot import _ntff_profile_via_ctypes
        hook = _ntff_profile_via_ctypes('/opt/axon/libaxon_pjrt.so')
        if hook is None:
            return
        import antenv
        mod = types.ModuleType('antenv.axon_hooks')
        mod.get_axon_ntff_profile_hook = lambda: hook
        mod.set_axon_ntff_profile_hook = lambda h: None
        sys.modules['antenv.axon_hooks'] = mod
        antenv.axon_hooks = mod
    except Exception:
        pass


def _safe_upload_artifacts():
    """upload_artifacts needs a bucket; make it degrade to a no-op path
    so tracing works in sandboxes without one."""
    try:
        from concourse import bass_utils
        orig = bass_utils.upload_artifacts
        if getattr(bass_utils, "_ul_wrapped", False):
            return
        def wrapped(tmpdir):
            try:
                return orig(tmpdir)
            except Exception:
                return str(tmpdir)
        bass_utils.upload_artifacts = wrapped
        bass_utils._ul_wrapped = True
    except Exception:
        pass


def _build_nc(reps=1):
    import concourse.bacc as bacc
    import concourse.mybir as mybir
    import concourse.tile as tile

    dt = mybir.dt
    f32 = dt.float32
    bf16 = dt.bfloat16
    nc = bacc.Bacc("TRN2", target_bir_lowering=False, debug=False)

    # per-chunk rows: [weight cols | mask cols]; fp8dr interleaves two
    # 128-row contraction planes per chunk for DoubleRow (K=256/matmul)
    mdt = dt.float8e4 if FP8 else bf16
    CW = 2 * (YC + FT) if FP8 else YC + FT
    WOFF = 2 * YC if FP8 else YC
    wm_d = nc.dram_tensor("wm", (KCC * 128, CW), mdt,
                          kind="ExternalInput")
    aux_d = nc.dram_tensor("aux", (128, TT), bf16, kind="ExternalInput")
    out_d = nc.dram_tensor("out", (YC, NCB), f32, kind="ExternalOutput")
    marg_d = nc.dram_tensor("marg", (YC, NCB), bf16, kind="ExternalOutput")

    with tile.TileContext(nc) as tc:
        with tc.tile_pool(name="const", bufs=1) as cpool, \
             tc.tile_pool(name="wp", bufs=1) as wpool, \
             tc.tile_pool(name="ps", bufs=8, space="PSUM") as ps, \
             tc.tile_pool(name="uz", bufs=8) as uzpool, \
             tc.tile_pool(name="sm", bufs=8) as smpool, \
             tc.tile_pool(name="po", bufs=8) as popool:
            junk_sb = cpool.tile([128, 512], bf16, tag="junk")
            nc.vector.memset(junk_sb, 1.0)
            neg1_sb = cpool.tile([128, 1], f32, tag="neg1")
            nc.vector.memset(neg1_sb, -1.0)

            for rep in range(reps):
                # PE warmup: junk matmuls from the earliest post-preamble
                # moment. The HAM full-speed grant trails the onset of
                # sustained PE activity by ~4-5us, so starting early pulls
                # the whole matmul stream onto the fast clock; the warmups
                # also bridge the chunk-0 DMA wait without a ramp reset.
                warm_pt = ps.tile([128, 512], f32, tag="pt", name="warm_pt")
                for _ in range(12):
                    nc.tensor.matmul(warm_pt, junk_sb[:, 0:128], junk_sb[:],
                                     start=True, stop=True)

                # combined weight|mask chunks, resident; chunk 0 issued
                # first since it gates the first real matmul. Whole-chunk
                # descriptors arrive soonest (splitting adds per-descriptor
                # latency); stripe across both HWDGE queues so aggregate
                # bandwidth covers the k-outer pace.
                wm_tiles = [wpool.tile([128, CW], mdt, tag=f"wm{k}",
                                       name=f"wm{k}")
                            for k in range(KCC)]
                for k in range(KCC):
                    eng = nc.sync if k % 2 == 0 else nc.scalar
                    eng.dma_start(out=wm_tiles[k],
                                  in_=wm_d.ap()[k * 128:(k + 1) * 128, :])
                aux_sb = cpool.tile([128, TT], bf16, tag="aux")
                nc.sync.dma_start(out=aux_sb, in_=aux_d.ap())
                revt_sb = aux_sb[:, 0:TT]

                rm_tiles = [smpool.tile([128, NCB], bf16, tag="rm",
                                        name=f"rm{yt}")
                            for yt in range(NYT)]
                mg_tiles = [smpool.tile([128, NCB], bf16, tag="mg",
                                        name=f"mg{yt}")
                            for yt in range(NYT)]

                def emit_mm(pt, k, yt, fs, fe):
                    wmt = wm_tiles[k]
                    if FP8:
                        lhsT = wmt[:, 0:2 * YC] \
                            .rearrange("p (i y) -> p i y", i=2) \
                            [:, :, yt * 128:(yt + 1) * 128]
                        rhs = wmt[:, 2 * YC:CW] \
                            .rearrange("p (i c) -> p i c", i=2)[:, :, fs:fe]
                        nc.tensor.matmul(
                            pt, lhsT, rhs,
                            start=(k == 0), stop=(k == KCC - 1),
                            perf_mode=mybir.MatmulPerfMode.DoubleRow)
                    else:
                        nc.tensor.matmul(pt,
                                         wmt[:, yt * 128:(yt + 1) * 128],
                                         wmt[:, YC + fs:YC + fe],
                                         start=(k == 0), stop=(k == KCC - 1))

                def emit_post(pt, yt, cs, width):
                    # cs: first n-col in rm/mg tiles; width: n count
                    nelem = width * TT
                    u_t = uzpool.tile([128, 512], bf16, tag="u")
                    u = u_t[:, 0:nelem]
                    nc.scalar.activation(u, pt,
                                         mybir.ActivationFunctionType.Copy,
                                         bias=-WSCALE)
                    b_t = uzpool.tile([128, 512], bf16, tag="b")
                    b = b_t[:, 0:nelem]
                    nc.vector.tensor_scalar(b, u, 0.0, None,
                                            mybir.AluOpType.is_gt)
                    z_t = uzpool.tile([128, 512], bf16, tag="z")
                    z = z_t[:, 0:nelem]
                    r_b = revt_sb.unsqueeze(1).broadcast_to((128, width, TT))
                    nc.vector.tensor_tensor(
                        z.rearrange("p (n t) -> p n t", n=width),
                        b.rearrange("p (n t) -> p n t", n=width),
                        r_b, mybir.AluOpType.mult)
                    nc.vector.tensor_reduce(
                        rm_tiles[yt][:, cs:cs + width],
                        z.rearrange("p (n t) -> p n t", n=width),
                        axis=mybir.AxisListType.X, op=mybir.AluOpType.max)
                    nc.vector.tensor_reduce(
                        mg_tiles[yt][:, cs:cs + width],
                        u.rearrange("p (n t) -> p n t", n=width),
                        axis=mybir.AxisListType.X, op=mybir.AluOpType.min,
                        apply_absolute_value=True)

                def emit_tail(yt):
                    tmp_t = popool.tile([128, NCB], bf16, tag="tmp")
                    nc.scalar.activation(tmp_t, rm_tiles[yt],
                                         mybir.ActivationFunctionType.Relu,
                                         bias=neg1_sb[:])
                    out_t = popool.tile([128, NCB], f32, tag="pout")
                    nc.scalar.activation(out_t, tmp_t,
                                         mybir.ActivationFunctionType.Copy,
                                         bias=float(T), scale=-1.0)
                    nc.scalar.dma_start(
                        out=marg_d.ap()[yt * 128:(yt + 1) * 128, :],
                        in_=mg_tiles[yt])
                    nc.sync.dma_start(
                        out=out_d.ap()[yt * 128:(yt + 1) * 128, :],
                        in_=out_t)

                # f0 pass: k-outer so the PE trails the mask builder
                # without stalling; all 8 banks accumulate in parallel.
                pts = []
                for k in range(KCC):
                    for yt in range(NYT):
                        if k == 0:
                            pts.append(ps.tile([128, FH], f32, tag="pt",
                                               name=f"pt0_{yt}"))
                        emit_mm(pts[yt], k, yt, 0, FH)
                for yt in range(NYT):
                    emit_post(pts[yt], yt, 0, NPF)

                # f1 pass: masks resident -> y-outer so banks finish
                # staggered and postproc overlaps later y-tiles. The last
                # y-tile runs as two 256-col halves to shorten the final
                # exposed drain chain.
                for yt in range(NYT - 1):
                    pt = ps.tile([128, FH], f32, tag="pt", name=f"pt1_{yt}")
                    for k in range(KCC):
                        emit_mm(pt, k, yt, FH, 2 * FH)
                    emit_post(pt, yt, NPF, NPF)
                    emit_tail(yt)
                yt = NYT - 1
                for half in range(2):
                    pth = ps.tile([128, FH // 2], f32, tag="pt",
                                  name=f"pt1h{half}")
                    fs = FH + half * (FH // 2)
                    for k in range(KCC):
                        emit_mm(pth, k, yt, fs, fs + FH // 2)
                    emit_post(pth, yt, NPF + half * 4, 4)
                emit_tail(yt)

    nc.compile()
    return nc


def _make_in_maps(inputs):
    import ml_dtypes

    input = np.ascontiguousarray(np.asarray(inputs["input"], dtype=np.float32))
    weight = np.ascontiguousarray(np.asarray(inputs["weight"], dtype=np.float32))

    # mask[x, n*TT+t] = (t >= s[n, x]); {0,1} is exact in bf16. t=63 is
    # dropped: a first crossing there yields out=64, same as none.
    tgrid = np.arange(TT, dtype=np.float32)
    REVT = np.tile(np.float32(T) - tgrid, (128, 1)).astype(ml_dtypes.bfloat16)

    mmdt = ml_dtypes.float8_e4m3 if FP8 else ml_dtypes.bfloat16

    def plane_pack(a):
        # rows x -> (chunk kk, plane i, partition p); fp8dr packs the two
        # 128-row planes of each K=256 chunk side by side per partition
        if not FP8:
            return a
        c = a.shape[1]
        return np.ascontiguousarray(
            a.reshape(KCC, 2, 128, c).transpose(0, 2, 1, 3)
            .reshape(KCC * 128, 2 * c))

    w_halves = [
        plane_pack((weight[:, yb * YC:(yb + 1) * YC] * WSCALE).astype(mmdt))
        for yb in range(Y_SH)
    ]
    mask_quads = []
    for nb in range(N_SH):
        insl = input[nb * NCB:(nb + 1) * NCB, :]          # (NCB, X)
        m = (tgrid[None, None, :] >= insl[:, :, None])    # (NCB, X, TT)
        m = m.transpose(1, 0, 2).reshape(X, FT).astype(mmdt)
        mask_quads.append(plane_pack(m))

    in_maps = []
    for c in range(8):
        yb, nb = c % Y_SH, c // Y_SH
        wm = np.concatenate([w_halves[yb], mask_quads[nb]], axis=1)
        in_maps.append({"wm": np.ascontiguousarray(wm),
                        "aux": np.ascontiguousarray(REVT)})
    return in_maps


def kernel(input, weight, t_series, T=64, **unused):
    global LAST_RESULTS
    from concourse import bass_utils

    _ensure_ntff_hook()
    _safe_upload_artifacts()
    if "nc" not in _cache:
        _cache["nc"] = _build_nc()
    nc = _cache["nc"]

    _cache["t_series"] = np.asarray(t_series, dtype=np.float32).reshape(-1)
    in_maps = _make_in_maps({"input": input, "weight": weight})

    res = bass_utils.run_bass_kernel_spmd(
        nc, in_maps, core_ids=list(range(8)), trace=TRACE)
    LAST_RESULTS = res

    O = np.empty((YY, NN), dtype=np.float32)
    M = np.empty((YY, NN), dtype=np.float32)
    for c, r in enumerate(res.results):
        yb, nb = c % Y_SH, c // Y_SH
        O[yb * YC:(yb + 1) * YC, nb * NCB:(nb + 1) * NCB] = r["out"]
        M[yb * YC:(yb + 1) * YC, nb * NCB:(nb + 1) * NCB] = \
            np.asarray(r["marg"]).astype(np.float32)
    out = np.ascontiguousarray(O.T)

    _host_fixup(out, M.T, np.asarray(input, np.float32),
                np.asarray(weight, np.float32))
    return out


def _host_fixup(out, margin, input, weight):
    """Recompute exactly (fp64) every element whose bf16 |V-1| margin is
    within the bf16 matmul error bound; in-place on `out`."""
    flags = margin < FIX_EPS * WSCALE   # device margin is in scaled-V units
    if not flags.any():
        return
    # first step index j with t_series[j] >= in; == T means never spikes
    s = np.searchsorted(_cache.get("t_series", np.arange(T, dtype=np.float32)),
                        input, side="left").astype(np.int64)
    s = np.clip(s, 0, T)
    w64 = weight.astype(np.float64)
    for n in np.unique(np.nonzero(flags)[0]):
        ys = np.nonzero(flags[n])[0]
        d = np.zeros((T + 1, len(ys)))
        np.add.at(d, s[n], w64[:, ys])           # scatter rows by spike step
        V = np.cumsum(d[:T], axis=0)
        c = V > 1.0
        any_c = c.any(axis=0)
        idx = np.argmax(c, axis=0)
        out[n, ys] = np.where(any_c, idx + 1, T).astype(np.float32)
